# revision 5
# baseline (speedup 1.0000x reference)
"""Trainium2 Bass kernel for nn_Attn_17738214933129.

Dense transformer attention block:
  Q/K/V projections from n_loc=2048 -> feat=512 (8 heads x 64),
  structural-bias softmax added to scaled QK^T scores, softmax, PV,
  output projection back to n_loc=2048.

Sharding: data-parallel over batch (16 -> 2 per core) across 8 NeuronCores,
weights replicated, no collectives.

Layout strategy (per core, rows = 2*512 = 1024):
  - q/k are uploaded pre-transposed/pre-tiled in fp8e4; Q/K projections run
    as fp8 DoubleRow matmuls with weights host-scaled by 64; the PSUM->SBUF
    copy descales (and folds the 1/DH for Q).
  - K projection is i-major (streams k-data), Q projection is ft-major over
    resident q-data prefetched during K, so QT[0] completes early and the
    attention pipeline (scores -> exp -> esm-mult) starts right after.
  - The V projection is split into 4 row-passes of one 2-bank PSUM pair
    each and interleaved, per (head-pair, batch) iteration, with the
    transposed-score matmuls, the scalar-engine exp and the DVE esm
    multiply.  This hides the ~34us of exp work under PE work instead of
    serializing a vector/scalar-bound attention phase after the
    projections.
  - Scores use the TRANSPOSED layout ST[k, q] per head; adjacent score
    matmuls alternate the 64-partition head halves (row groups 0-1 vs 2-3)
    so the PE packs them concurrently via tile_position row tiling.
  - V carries 64 ones-columns per head so PV rows 0-63 hold the softmax
    row-sum; the reciprocal runs wide on DVE and the normalization fuses
    into a tensor-tensor multiply into the xT staging tiles.
  - PV+normalize for batch 0 runs inside the merged phase (its V passes
    complete b=0 first); the post phase runs PV(b=1) interleaved with the
    output projection.
  - DMA dispatch (~0.6us per dma_start on the issuing sequencer) is spread
    across the sync/scalar/vector/gpsimd queues.
"""

import sys

import numpy as np

try:
    import concourse.bass as bass  # noqa: F401
except Exception:  # pragma: no cover - path fallback
    sys.path.insert(0, "/opt/trn_rl_repo")

import ml_dtypes

import concourse.bacc as bacc
import concourse.tile as tile
from concourse import mybir
from concourse.bass_utils import run_bass_kernel_spmd

BF16 = mybir.dt.bfloat16
FP8 = mybir.dt.float8e4
F32 = mybir.dt.float32
AF = mybir.ActivationFunctionType
ALU = mybir.AluOpType
DR = mybir.MatmulPerfMode.DoubleRow

B, S, NLOC = 16, 512, 2048
FEAT, H, DH = 512, 8, 64
NCORES = 8
BL = B // NCORES          # batch per core = 2
R = BL * S                # rows per core = 1024
KT_N = NLOC // 128        # 16 contraction tiles for projections
FT_N = FEAT // 128        # 4 feature tiles
QT_N = S // 128           # 4 query tiles per batch element
WS = 64.0                 # host weight scale for fp8 Q/K weights

_CACHE = {}


def _build(use_bias):
    nc = bacc.Bacc(
        "TRN2",
        target_bir_lowering=False,
        debug=False,
        enable_asserts=False,
        num_devices=NCORES,
    )

    # q/k pre-transposed and pre-tiled on host: [128, i*R + r] = x[r, i*128+p].
    d_q = nc.dram_tensor("q", [128, KT_N * R], FP8, kind="ExternalInput").ap()
    d_k = nc.dram_tensor("k", [128, KT_N * R], FP8, kind="ExternalInput").ap()
    d_v = nc.dram_tensor("v", [128, KT_N * R], BF16, kind="ExternalInput").ap()
    # esmT = exp(softmax(masked str_mat))^T, pre-tiled (computed on host):
    # [128, (b*4+kt)*512 + q] = exp(sm)[b, q, kt*128+p].
    d_sm = nc.dram_tensor("smh", [128, BL * QT_N * S], BF16, kind="ExternalInput").ap()
    # weights pre-tiled: wq/wk/wv [128, 16*512] with [p, i*512+f]=W.T[i*128+p, f];
    # wo [128, 4*2048] with [p, ft*2048+n]=Wo.T[ft*128+p, n].
    d_wq = nc.dram_tensor("wqT", [128, KT_N * FEAT], FP8, kind="ExternalInput").ap()
    d_wk = nc.dram_tensor("wkT", [128, KT_N * FEAT], FP8, kind="ExternalInput").ap()
    d_wv = nc.dram_tensor("wvT", [128, KT_N * FEAT], BF16, kind="ExternalInput").ap()
    d_wo = nc.dram_tensor("woT", [128, FT_N * NLOC], BF16, kind="ExternalInput").ap()
    d_bq = nc.dram_tensor("bqr", [1, FEAT], BF16, kind="ExternalInput").ap()
    d_bk = nc.dram_tensor("bkr", [1, FEAT], BF16, kind="ExternalInput").ap()
    d_bv = nc.dram_tensor("bvr", [1, FEAT], BF16, kind="ExternalInput").ap()
    d_bo = nc.dram_tensor("bor", [1, NLOC], BF16, kind="ExternalInput").ap()
    d_ones = nc.dram_tensor("onesr", [1, 512], BF16, kind="ExternalInput").ap()
    d_out = nc.dram_tensor("out", [R, NLOC], BF16, kind="ExternalOutput").ap()

    dv3 = d_v.rearrange("p (i r) -> p i r", r=R)

    with tile.TileContext(nc) as tc:
        with (
            tc.tile_pool(name="consts", bufs=1) as cpool,
            tc.tile_pool(name="weights", bufs=1) as wpool,
            tc.tile_pool(name="persist", bufs=1) as ppool,
            tc.tile_pool(name="kstream", bufs=3) as kpool,
            tc.tile_pool(name="qres", bufs=1) as qpool,
            tc.tile_pool(name="vstream", bufs=4) as vtpool,
            tc.tile_pool(name="esbuf", bufs=3) as espool,
            tc.tile_pool(name="etbuf", bufs=1) as etpool,
            tc.tile_pool(name="cols", bufs=2) as colpool,
            tc.tile_pool(name="ostage", bufs=2) as opool,
            tc.tile_pool(name="psumA", bufs=2, space="PSUM") as psA,
            tc.tile_pool(name="psumB", bufs=2, space="PSUM") as psB,
        ):
            def dma_psplit(dst, src, parts=4, eng=None):
                """Issue a DMA as `parts` partition-range slices (each slice
                still shards across the 16 HW queues; splitting mainly cuts
                per-dispatch latency for first-use tiles)."""
                step = dst.shape[0] // parts
                for j in range(parts):
                    (eng or nc.sync).dma_start(
                        dst[j * step : (j + 1) * step], src[j * step : (j + 1) * step]
                    )

            ones = cpool.tile([1, 512], BF16, tag="ones", name="ones")
            biases = {}
            if use_bias:
                for nm, dr, width in (
                    ("bq", d_bq, FEAT),
                    ("bk", d_bk, FEAT),
                    ("bv", d_bv, FEAT),
                    ("bo", d_bo, NLOC),
                ):
                    t = cpool.tile([1, width], BF16, tag=nm, name=nm)
                    nc.gpsimd.dma_start(t[:], dr[:])
                    biases[nm] = t

            # Persistent activations.
            QT = [ppool.tile([128, R], BF16, tag=f"QT{i}", name=f"QT{i}") for i in range(FT_N)]
            KTt = [ppool.tile([128, R], BF16, tag=f"KT{i}", name=f"KT{i}") for i in range(FT_N)]
            V6 = ppool.tile([128, R // 128, H, 2 * DH], BF16, tag="V6", name="V6")
            xT = [
                [ppool.tile([128, S], BF16, tag=f"xT{b}{j}", name=f"xT{b}{j}") for j in range(FT_N)]
                for b in range(BL)
            ]
            sm_t = [
                ppool.tile([128, QT_N * S], BF16, tag=f"smh{b}", name=f"smh{b}")
                for b in range(BL)
            ]

            wq = wpool.tile([128, KT_N, FEAT], FP8, tag="wq", name="wq")
            wk = wpool.tile([128, KT_N, FEAT], FP8, tag="wk", name="wk")
            wv = wpool.tile([128, KT_N, FEAT], BF16, tag="wv", name="wv")
            wo = wpool.tile([128, FT_N, NLOC], BF16, tag="wo", name="wo")

            # ---- early DMA program ------------------------------------------
            # sync: k-data stream (first tile split for latency)
            # scalar: wk then wq
            # vector: ones, sm, wv, wo
            # gpsimd: resident q-data prefetch, V6 ones-memset
            xk = [kpool.tile([128, 2, R], FP8, tag="xk", name=f"xk{i}") for i in range(2)]
            nc.sync.dma_start(xk[0][0:64], d_k[0:64, 0 : 2 * R])
            nc.gpsimd.dma_start(xk[0][64:128], d_k[64:128, 0 : 2 * R])
            dma_psplit(wk[:, 0:4, :], d_wk[:, 0 : 4 * FEAT], parts=4, eng=nc.scalar)
            dma_psplit(xk[1][:], d_k[:, 2 * R : 4 * R], parts=2)
            nc.gpsimd.dma_start(ones[:], d_ones[:])
            for i in range(2, KT_N // 2):
                t = kpool.tile([128, 2, R], FP8, tag="xk", name=f"xk{i}")
                nc.sync.dma_start(t[:], d_k[:, 2 * i * R : 2 * (i + 1) * R])
                xk.append(t)
            for c in range(1, 4):
                nc.scalar.dma_start(
                    wk[:, 4 * c : 4 * (c + 1), :],
                    d_wk[:, 4 * c * FEAT : 4 * (c + 1) * FEAT],
                )
            # q-data resident prefetch on gpsimd queue
            xq = []
            for i in range(KT_N // 2):
                t = qpool.tile([128, 2, R], FP8, tag=f"xq{i}", name=f"xq{i}")
                nc.gpsimd.dma_start(t[:], d_q[:, 2 * i * R : 2 * (i + 1) * R])
                xq.append(t)
            nc.gpsimd.memset(V6[:, :, :, 0:DH], 1.0)
            for b in range(BL):
                nc.gpsimd.dma_start(
                    sm_t[b][:], d_sm[:, b * QT_N * S : (b + 1) * QT_N * S]
                )
            dma_psplit(wq[:], d_wq[:], parts=2, eng=nc.scalar)
            dma_psplit(wv[:], d_wv[:], parts=4, eng=nc.gpsimd)
            dma_psplit(wo[:], d_wo[:], parts=2, eng=nc.scalar)

            def bias_init(ps, bias_nm):
                for rc in range(2):
                    nc.tensor.matmul(
                        ps[:, rc, :],
                        lhsT=biases[bias_nm][0:1, 0:128],
                        rhs=ones[0:1, :],
                        start=True,
                        stop=False,
                    )

            # ---- K projection: i-major fp8 DoubleRow ------------------------
            kg = []
            for ft in range(FT_N):
                pool = psA if ft < 2 else psB
                ps = pool.tile([128, 2, 512], F32, tag=pool.name, name="kps")
                if use_bias:
                    for rc in range(2):
                        nc.tensor.matmul(
                            ps[:, rc, :],
                            lhsT=biases["bk"][0:1, ft * 128 : (ft + 1) * 128],
                            rhs=ones[0:1, :],
                            start=True,
                            stop=False,
                        )
                kg.append(ps)
            for i in range(KT_N // 2):
                for ft in range(FT_N):
                    for rc in range(2):
                        nc.tensor.matmul(
                            kg[ft][:, rc, :],
                            lhsT=wk[:, 2 * i : 2 * i + 2, ft * 128 : (ft + 1) * 128],
                            rhs=xk[i][:, :, rc * 512 : (rc + 1) * 512],
                            start=(i == 0 and not use_bias),
                            stop=(i == KT_N // 2 - 1),
                            perf_mode=DR,
                        )
            for ft in range(FT_N):
                nc.scalar.mul(KTt[ft][:, 0:R], kg[ft][:, :, :], 1.0 / WS)

            # ---- Q projection: ft-major over resident q-data ----------------
            for ft in range(FT_N):
                pool = psA if ft % 2 == 0 else psB
                ps = pool.tile([128, 2, 512], F32, tag=pool.name, name="qps")
                if use_bias:
                    for rc in range(2):
                        nc.tensor.matmul(
                            ps[:, rc, :],
                            lhsT=biases["bq"][0:1, ft * 128 : (ft + 1) * 128],
                            rhs=ones[0:1, :],
                            start=True,
                            stop=False,
                        )
                for i in range(KT_N // 2):
                    for rc in range(2):
                        nc.tensor.matmul(
                            ps[:, rc, :],
                            lhsT=wq[:, 2 * i : 2 * i + 2, ft * 128 : (ft + 1) * 128],
                            rhs=xq[i][:, :, rc * 512 : (rc + 1) * 512],
                            start=(i == 0 and not use_bias),
                            stop=(i == KT_N // 2 - 1),
                            perf_mode=DR,
                        )
                nc.scalar.mul(QT[ft][:, 0:R], ps[:, :, :], 1.0 / (WS * DH))

            # ---- merged phase: V passes + scores/exp/mult + PV(b=0) ---------
            ET = {}

            def pv_block(b, hp, itd):
                """PV + reciprocal + normalize into xT[b][hp] from ET[itd]."""
                yp = psA.tile([128, 2, 512], F32, tag=psA.name, name="yp")
                for hs in range(2):
                    h = 2 * hp + hs
                    for kt in range(QT_N):
                        nc.tensor.matmul(
                            yp[:, hs, :],
                            lhsT=V6[:, b * QT_N + kt, h, :],
                            rhs=ET[itd][hs][:, kt, :],
                            start=(kt == 0),
                            stop=(kt == QT_N - 1),
                        )
                rs2 = colpool.tile([64, 2, S], F32, tag="rs2", name="rs2")
                nc.vector.reciprocal_approx_fast(rs2[:], yp[0:DH, :, :])
                for hs in range(2):
                    hb = hs * 64
                    nc.vector.tensor_tensor(
                        xT[b][hp][hb : hb + 64, :],
                        yp[DH : 2 * DH, hs, :],
                        rs2[:, hs, :],
                        op=ALU.mult,
                    )

            vp = None
            for it in range(8):
                hp, b = it // 2, it % 2
                # scores (packed row groups) -> exp -> esm multiply
                ET[it] = {
                    hs: etpool.tile(
                        [128, QT_N, S], BF16, tag=f"et{b}", bufs=(6 if b == 0 else 8),
                        name=f"ET{it}_{hs}",
                    )
                    for hs in range(2)
                }
                for jj in range(2):
                    P = {
                        hs: psB.tile([128, 2, 512], F32, tag=psB.name, name="sps")
                        for hs in range(2)
                    }
                    for kt in (2 * jj, 2 * jj + 1):
                        for hs in range(2):
                            hb = hs * 64
                            nc.tensor.matmul(
                                P[hs][:, kt % 2, :],
                                lhsT=KTt[hp][
                                    hb : hb + 64,
                                    b * S + kt * 128 : b * S + (kt + 1) * 128,
                                ],
                                rhs=QT[hp][hb : hb + 64, b * S : (b + 1) * S],
                                start=True,
                                stop=True,
                            )
                    for hs in range(2):
                        es = espool.tile([128, 2, S], BF16, tag="es", name="es")
                        nc.scalar.activation(es[:], P[hs][:], AF.Exp)
                        nc.vector.tensor_tensor(
                            ET[it][hs][:, 2 * jj : 2 * jj + 2, :],
                            es[:],
                            sm_t[b][:, 2 * jj * S : (2 * jj + 2) * S],
                            op=ALU.mult,
                        )
                # PV for batch 0 (V6 rows for b=0 complete after pass 1 = it 3)
                if it >= 4:
                    pv_block(0, it - 4, 2 * (it - 4))
                # V projection chunk: pass p covers rows [256p, 256p+256),
                # halves of the 16 k-subtiles per it.
                p, lh = it // 2, it % 2
                if lh == 0:
                    vp = psA.tile([128, 2, 512], F32, tag=psA.name, name="vps")
                    if use_bias:
                        for rtl in range(2):
                            nc.tensor.matmul(
                                vp[:, rtl, :],
                                lhsT=ones[0:1, 0:128],
                                rhs=biases["bv"][0:1, :],
                                start=True,
                                stop=False,
                            )
                for ii in range(8 * lh, 8 * lh + 8, 2):
                    vt = vtpool.tile([128, 2, 256], BF16, tag="vt", name="vt")
                    nc.gpsimd.dma_start(
                        vt[:], dv3[:, ii : ii + 2, 256 * p : 256 * p + 256]
                    )
                    for io in range(2):
                        i = ii + io
                        for rtl in range(2):
                            nc.tensor.matmul(
                                vp[:, rtl, :],
                                lhsT=vt[:, io, rtl * 128 : (rtl + 1) * 128],
                                rhs=wv[:, i, :],
                                start=(i == 0 and not use_bias),
                                stop=(i == KT_N - 1),
                            )
                if lh == 1:
                    # rows [256p, 256p+256) -> V6[:, 2p:2p+2, :, DH:2DH]
                    nc.vector.tensor_copy(
                        V6[:, 2 * p : 2 * p + 2, :, DH : 2 * DH],
                        vp[:].rearrange("p t (h d) -> p t h d", h=H),
                    )

            # ---- post phase: PV(b=1) interleaved with output projection ----
            def out_block(b, qt):
                row0 = b * S + qt * 128
                ot = opool.tile([128, NLOC], BF16, tag="ot", name="ot")
                for j in range(2):
                    ps = psB.tile([128, 2, 512], F32, tag=psB.name, name="ops")
                    for half in range(2):
                        nlc = 2 * j + half
                        if use_bias:
                            nc.tensor.matmul(
                                ps[:, half, :],
                                lhsT=ones[0:1, 0:128],
                                rhs=biases["bo"][0:1, nlc * 512 : (nlc + 1) * 512],
                                start=True,
                                stop=False,
                            )
                        for ft in range(FT_N):
                            nc.tensor.matmul(
                                ps[:, half, :],
                                lhsT=xT[b][ft][:, qt * 128 : (qt + 1) * 128],
                                rhs=wo[:, ft, nlc * 512 : (nlc + 1) * 512],
                                start=(ft == 0 and not use_bias),
                                stop=(ft == FT_N - 1),
                            )
                    dst = ot[:, 2 * j * 512 : (2 * j + 2) * 512]
                    if j == 0:
                        nc.scalar.copy(dst, ps[:, :, :])
                    else:
                        nc.vector.tensor_copy(dst, ps[:, :, :])
                nc.sync.dma_start(d_out[row0 : row0 + 128, :], ot[:])

            for hp in range(QT_N):
                pv_block(1, hp, 2 * hp + 1)
                out_block(0, hp)
            for qt in range(QT_N):
                out_block(1, qt)

    nc.compile()
    return nc


def _prep_inputs(q, k, v, str_mat, attn_mask, Wq, bq, Wk, bk, Wv, bv, Wo, bo):
    bf = ml_dtypes.bfloat16
    f8 = ml_dtypes.float8_e4m3
    # fp8 Q/K weights host-scaled by WS=64 to stay in normal range; the
    # PSUM copy-out divides it back (and folds 1/DH for Q).
    wqT = np.ascontiguousarray((Wq * np.float32(WS)).T).astype(f8)
    wkT = np.ascontiguousarray((Wk * np.float32(WS)).T).astype(f8)
    wvT = np.ascontiguousarray(Wv.T).astype(bf)
    woT = np.ascontiguousarray(Wo.T).astype(bf)

    # Pre-tile weights: [n*128, width] -> [128, n*width].
    def pretile(w):
        n = w.shape[0] // 128
        return np.ascontiguousarray(
            w.reshape(n, 128, w.shape[1]).transpose(1, 0, 2).reshape(128, -1)
        )

    wqt = pretile(wqT)
    wkt = pretile(wkT)
    wvt = pretile(wvT)
    wot = pretile(woT)

    bqr = (bq[None, :] * np.float32(WS / DH)).astype(bf)
    bkr = (bk[None, :] * np.float32(WS)).astype(bf)
    bvr = bv[None, :].astype(bf)
    bor = bo[None, :].astype(bf)
    onesr = np.ones((1, 512), dtype=bf)

    q8 = np.asarray(q).astype(f8)
    k8 = np.asarray(k).astype(f8)
    v16 = np.asarray(v).astype(bf)

    def pretile_T(x):
        # [R, NLOC] -> [128, KT_N*R] with [p, i*R+r] = x[r, i*128+p]
        return np.ascontiguousarray(
            x.reshape(R, KT_N, 128).transpose(2, 1, 0).reshape(128, KT_N * R)
        )

    # Structural softmax on host; upload exp of its TRANSPOSE in bf16.
    strf = np.asarray(str_mat, dtype=np.float32)
    maskf = np.asarray(attn_mask)
    sm = np.where(maskf == 0, np.float32(-1e9), strf)
    sm = sm - sm.max(-1, keepdims=True)
    np.exp(sm, out=sm)
    sm /= sm.sum(-1, keepdims=True)
    smT16 = np.exp(np.ascontiguousarray(sm.transpose(0, 2, 1))).astype(bf)

    in_maps = []
    for c in range(NCORES):
        sl = slice(c * BL, (c + 1) * BL)
        # [BL, S(k), S(q)] -> [128, BL*QT_N*S] with [p, (b*4+kt)*S+q].
        smt = np.ascontiguousarray(
            smT16[sl].reshape(BL * QT_N, 128, S).transpose(1, 0, 2).reshape(128, -1)
        )
        in_maps.append(
            {
                "q": pretile_T(q8[sl].reshape(R, NLOC)),
                "k": pretile_T(k8[sl].reshape(R, NLOC)),
                "v": pretile_T(v16[sl].reshape(R, NLOC)),
                "smh": smt,
                "wqT": wqt,
                "wkT": wkt,
                "wvT": wvt,
                "woT": wot,
                "bqr": bqr,
                "bkr": bkr,
                "bvr": bvr,
                "bor": bor,
                "onesr": onesr,
            }
        )
    return in_maps


def kernel(q, k, v, str_mat, attn_mask, Wq, bq, Wk, bk, Wv, bv, Wo, bo):
    use_bias = bool(
        np.any(np.asarray(bq))
        or np.any(np.asarray(bk))
        or np.any(np.asarray(bv))
        or np.any(np.asarray(bo))
    )
    key = ("nc", use_bias)
    if key not in _CACHE:
        _CACHE[key] = _build(use_bias)
    nc = _CACHE[key]
    in_maps = _prep_inputs(
        q, k, v, str_mat, attn_mask, Wq, bq, Wk, bk, Wv, bv, Wo, bo
    )
    res = run_bass_kernel_spmd(nc, in_maps, core_ids=list(range(NCORES)))
    out = np.empty((B, S, NLOC), dtype=np.float32)
    for c in range(NCORES):
        out[c * BL : (c + 1) * BL] = (
            res.results[c]["out"].astype(np.float32).reshape(BL, S, NLOC)
        )
    return out
